# revision 6
# baseline (speedup 1.0000x reference)
"""Trainium2 Bass kernel for the LP contrastive loss.

loss = mean_b( -log( pos_min_b / (pos_min_b + neg_sum_b + 1e-6) + 1e-6 ) )
  with E = exp(feats @ fs.T / TEMP), pos/neg split by label equality.

Sharding: the support set (N = Bs*TOPK = 16384) is split across the 8
cores (2048 columns each); every core keeps the full query batch
B = 2048 and computes a [2048 x 2048] slice of the similarity matrix.
Host combines the per-core partials (min of mins, sum of sums) and
applies the final -log(...)/mean in float64.

fp8 DoubleRow matmul at the FLOP floor (8 k-pairs of 256 contraction
rows each, C=2048; the PE consumes 2 fp8 rows/cycle = 2x bf16), with
the label mask precomputed on the HOST into a bf16 input tensor
(-16384 at positives, 0 elsewhere), packed in compute order.

Per 128x512 tile:
  PE : 8 DoubleRow matmuls              (4096 cyc = 1.71 us @2.4GHz)
  DVE: v = ps + mask  (tensor_tensor)   (~0.70 us)
       row-min(v)     (tensor_reduce)   (~0.64 us)
  Act: exp((20/1024)*v), fused row-sum  (~0.47 us)
so the sweep is PE-bound at the fp8 roofline (109.2 us/core @2.4GHz;
the PE sustains ~1.95GHz under continuous load, ~137 us).  The timing
rep-loop emits 16 sweeps per For_i iteration: the loop back-edge
costs ~6 us/sweep otherwise (138.4 -> 131.5 us/rep measured).

PSUM holds gamma*s (gamma=1024, inputs pre-scaled by 32 before fp8
quantization); v = gamma*(s - 16*is_pos); positives underflow exp to 0.
Host combines cores (min of mins, sum of sums) in float64.
"""

import sys

sys.path.insert(0, "/opt/trn_rl_repo")

import numpy as np
import ml_dtypes

TEMP = 0.05
SCALE = 1.0 / TEMP  # 20.0
NCORES = 8
ALPHA = 32.0
GAMMA = ALPHA * ALPHA  # PSUM holds GAMMA * s
BIG = 16.0  # mask offset in s-units; mask value is -GAMMA*BIG = -16384

_CACHE = {}


def _build(B, C, Nsh, reps=1, unroll=1):
    import contextlib

    import concourse.tile as tile
    from concourse import bacc, mybir

    dt = mybir.dt
    MT = B // 128
    KT = C // 128  # 16 k-subtiles
    KP = KT // 2  # 8 DoubleRow pairs
    NT = Nsh // 512
    CH = 512
    MC = B // CH

    nc = bacc.Bacc("TRN2", target_bir_lowering=False, debug=False, num_devices=NCORES)

    featsL = nc.dram_tensor(
        "featsL", [128, MC, KT, CH], dt.float8e4, kind="ExternalInput"
    ).ap()
    fsL = nc.dram_tensor(
        "fsL", [128, NT, KT, 512], dt.float8e4, kind="ExternalInput"
    ).ap()
    # host-precomputed mask image, n-major consume order:
    # masksD[p, n, m, j] = -16384 if labels[m*128+p] == labels_s[n*512+j]
    masksD = nc.dram_tensor(
        "masksD", [128, NT, MT, 512], dt.bfloat16, kind="ExternalInput"
    ).ap()
    minv_d = nc.dram_tensor("minv", [128, MT], dt.float32, kind="ExternalOutput").ap()
    sums_d = nc.dram_tensor("sums", [128, MT], dt.float32, kind="ExternalOutput").ap()

    with tile.TileContext(nc) as tc:
        with (
            tc.tile_pool(name="res", bufs=1) as res,
            tc.tile_pool(name="work", bufs=4) as work,
            tc.tile_pool(name="ps", bufs=8, space="PSUM") as psum,
        ):
            # --- resident tiles, DMA'd in the order compute consumes them ---
            lhs_t = [None] * MC
            rhs_t = [None] * NT
            masks_t = res.tile([128, NT, MT, 512], dt.bfloat16, tag="masks")

            rhs_t[0] = res.tile([128, KT, 512], dt.float8e4, name="rhs0", tag="rhs0")
            nc.sync.dma_start(rhs_t[0][:], fsL[:, 0, :, :])
            lhs_t[0] = res.tile([128, KT, CH], dt.float8e4, name="lhs0", tag="lhs0")
            nc.sync.dma_start(lhs_t[0][:], featsL[:, 0, :, :])
            # first few masks of the n=0 sweep, then the rest of the lhs,
            # then the tail of the n=0 masks, then (rhs, masks) per later n.
            nc.sync.dma_start(masks_t[:, 0, 0:4, :], masksD[:, 0, 0:4, :])
            for c in range(1, MC):
                lhs_t[c] = res.tile(
                    [128, KT, CH], dt.float8e4, name=f"lhs{c}", tag=f"lhs{c}"
                )
                nc.sync.dma_start(lhs_t[c][:], featsL[:, c, :, :])
            nc.sync.dma_start(masks_t[:, 0, 4:MT, :], masksD[:, 0, 4:MT, :])
            for n in range(1, NT):
                rhs_t[n] = res.tile(
                    [128, KT, 512], dt.float8e4, name=f"rhs{n}", tag=f"rhs{n}"
                )
                nc.sync.dma_start(rhs_t[n][:], fsL[:, n, :, :])
                nc.sync.dma_start(masks_t[:, n, :, :], masksD[:, n, :, :])

            mincols = res.tile([128, MT, NT], dt.float32, tag="mincols")
            sumcols = res.tile([128, MT, NT], dt.float32, tag="sumcols")
            minv_t = res.tile([128, MT], dt.float32, tag="minv")
            sums_t = res.tile([128, MT], dt.float32, tag="sums")

            warm = res.tile([128, 512], dt.bfloat16, tag="warm")
            nc.vector.memset(warm[:], 0.0)
            wps = psum.tile([128, 512], dt.float32, tag="ps")
            for w in range(30):
                nc.tensor.matmul(
                    wps[:], warm[:, 0:128], warm[:], start=(w == 0), stop=(w == 29)
                )

            n_loop, n_flat = divmod(reps, unroll)
            rep_loop = (
                tc.For_i(
                    0,
                    n_loop,
                    1,
                    hint_engines=(mybir.EngineType.PE, mybir.EngineType.DVE),
                )
                if n_loop > 1 or (n_loop == 1 and n_flat)
                else contextlib.nullcontext()
            )
            with rep_loop:
              for _u in range(unroll if n_loop else 0):
                for n in range(NT):
                    for m in range(MT):
                        c, ci = divmod(m * 128, CH)
                        ps = psum.tile([128, 512], dt.float32, tag="ps")
                        for kk in range(KP):
                            nc.tensor.matmul(
                                ps[:],
                                lhs_t[c][:, 2 * kk : 2 * kk + 2, ci : ci + 128],
                                rhs_t[n][:, 2 * kk : 2 * kk + 2, :],
                                start=(kk == 0),
                                stop=(kk == KP - 1),
                                perf_mode=mybir.MatmulPerfMode.DoubleRow,
                            )
                        v_t = work.tile([128, 512], dt.float32, tag="v")
                        nc.vector.tensor_tensor(
                            v_t[:], ps[:], masks_t[:, n, m, :], mybir.AluOpType.add
                        )
                        nc.vector.tensor_reduce(
                            mincols[:, m, n : n + 1],
                            v_t[:],
                            axis=mybir.AxisListType.X,
                            op=mybir.AluOpType.min,
                        )
                        e_t = work.tile([128, 512], dt.float32, tag="e")
                        nc.scalar.activation(
                            e_t[:],
                            v_t[:],
                            mybir.ActivationFunctionType.Exp,
                            scale=SCALE / GAMMA,
                            accum_out=sumcols[:, m, n : n + 1],
                        )

            for _f in range(n_flat):
                for n in range(NT):
                    for m in range(MT):
                        c, ci = divmod(m * 128, CH)
                        ps = psum.tile([128, 512], dt.float32, tag="ps")
                        for kk in range(KP):
                            nc.tensor.matmul(
                                ps[:],
                                lhs_t[c][:, 2 * kk : 2 * kk + 2, ci : ci + 128],
                                rhs_t[n][:, 2 * kk : 2 * kk + 2, :],
                                start=(kk == 0),
                                stop=(kk == KP - 1),
                                perf_mode=mybir.MatmulPerfMode.DoubleRow,
                            )
                        v_t = work.tile([128, 512], dt.float32, tag="v")
                        nc.vector.tensor_tensor(
                            v_t[:], ps[:], masks_t[:, n, m, :], mybir.AluOpType.add
                        )
                        nc.vector.tensor_reduce(
                            mincols[:, m, n : n + 1],
                            v_t[:],
                            axis=mybir.AxisListType.X,
                            op=mybir.AluOpType.min,
                        )
                        e_t = work.tile([128, 512], dt.float32, tag="e")
                        nc.scalar.activation(
                            e_t[:],
                            v_t[:],
                            mybir.ActivationFunctionType.Exp,
                            scale=SCALE / GAMMA,
                            accum_out=sumcols[:, m, n : n + 1],
                        )

            nc.vector.tensor_reduce(
                minv_t[:], mincols[:], axis=mybir.AxisListType.X, op=mybir.AluOpType.min
            )
            nc.vector.tensor_reduce(
                sums_t[:], sumcols[:], axis=mybir.AxisListType.X, op=mybir.AluOpType.add
            )
            nc.sync.dma_start(minv_d[:], minv_t[:])
            nc.sync.dma_start(sums_d[:], sums_t[:])

    nc.compile()
    return nc


def get_nc(B, C, Nsh, reps=1, unroll=16):
    key = (B, C, Nsh, reps, unroll)
    if key not in _CACHE:
        _CACHE[key] = _build(B, C, Nsh, reps, unroll)
    return _CACHE[key]


def _pack(matT, nchunks, chunk, KT):
    return np.ascontiguousarray(
        matT.reshape(KT, 128, nchunks, chunk).transpose(1, 2, 0, 3)
    )


def make_in_maps(feats, feats_s, labels, labels_s):
    feats = np.asarray(feats, dtype=np.float32)
    fs = np.asarray(feats_s, dtype=np.float32).reshape(-1, feats.shape[1])
    labels = np.asarray(labels).astype(np.int64)
    labels_s = np.asarray(labels_s).astype(np.int64)

    B, C = feats.shape
    N = fs.shape[0]
    Nsh = N // NCORES
    KT = C // 128
    MT = B // 128
    NT = Nsh // 512
    fp8 = ml_dtypes.float8_e4m3

    bound = float(
        np.linalg.norm(feats, axis=1).max() * np.linalg.norm(fs, axis=1).max()
    )
    alpha = ALPHA / max(1.0, np.sqrt(bound))
    gamma = alpha * alpha
    # mask value must be exactly representable in bf16 so the host-side
    # un-offset (SCALE*big) matches what the DVE actually added
    maskval = float(ml_dtypes.bfloat16(-BIG * max(1.0, bound) * gamma))
    big = -maskval / gamma

    featsL = _pack((feats.T * alpha).astype(fp8), B // 512, 512, KT)
    fsT_all = (fs.T * alpha).astype(fp8)

    in_maps = []
    for i in range(NCORES):
        sl = slice(i * Nsh, (i + 1) * Nsh)
        fsL_i = _pack(fsT_all[:, sl], NT, 512, KT)
        # mask image [128, NT, MT, 512]: [p, n, m, j] for query m*128+p,
        # support col n*512+j of this core's shard
        is_pos = labels[:, None] == labels_s[None, sl]  # [B, Nsh]
        mask = np.where(is_pos, np.float32(maskval), np.float32(0.0))
        masksD = np.ascontiguousarray(
            mask.reshape(MT, 128, NT, 512).transpose(1, 2, 0, 3)
        ).astype(ml_dtypes.bfloat16)
        in_maps.append({"featsL": featsL, "fsL": fsL_i, "masksD": masksD})
    return in_maps, B, C, Nsh, (big, gamma)


def finish_on_host(results, B, big=(BIG, GAMMA)):
    bigv, gamma = big
    MT = B // 128
    minv = np.stack([r["minv"].T.reshape(B) for r in results])
    sums = np.stack([r["sums"].T.reshape(B) for r in results])
    vmin = minv.min(axis=0).astype(np.float64) / gamma
    neg_sum = sums.astype(np.float64).sum(axis=0)
    with np.errstate(over="ignore", invalid="ignore"):
        pos_min = np.exp(SCALE * vmin + SCALE * bigv)
        loss = -np.log(pos_min / (pos_min + neg_sum + 1e-6) + 1e-6)
    return np.float32(loss.mean())


def kernel(**inputs):
    from concourse.bass_utils import run_bass_kernel_spmd

    in_maps, B, C, Nsh, big = make_in_maps(
        inputs["feats"], inputs["feats_s"], inputs["labels"], inputs["labels_s"]
    )
    nc = get_nc(B, C, Nsh)
    res = run_bass_kernel_spmd(nc, in_maps, core_ids=list(range(NCORES)))
    return finish_on_host(res.results, B, big)
